# revision 4
# baseline (speedup 1.0000x reference)
"""Causal self-attention (RoPE) TRN2 kernel — head-tensor-parallel, fused.

Sharding (per spec hint): core m = (batch m//4, heads 4*(m%4)..+4). Each
core projects q/k/v for its 4 heads over the full sequence, runs causal
attention, and applies its row-slice of W_proj, producing a PARTIAL
output [C, T] f32. The host sums the 4 partials per batch (the
post-c_proj all-reduce) and transposes. No on-device collectives.

One fused instruction stream per core, software-pipelined so the PE
(tensor) engine never waits on the ACT exp chain or the DVE/Pool
softmax work:

  [v proj] [k/q proj h0] [attn h0 || k/q proj h1] [attn h1 || proj h2]
  [attn h2 || proj h3] [attn h3 || out-proj qtp0] [out-proj rest]

Head h's attention blocks are interleaved with head h+1's projection
matmuls (~1.6 proj MMs per attention block); the last head's attention
interleaves the first half of the output projection instead (gated on
the already-flushed y tiles; capped so no emitted matmul can wait on a
flush that is emitted later in the in-order PE queue).

On-chip layout is fully transposed (channels on partitions, tokens on
the free dim): qT/kT = [D, T] per head, v = [tokens, D], S^T = [keys, q]
so the PV matmul needs no transposes. RoPE: W_attn q/k columns are
permuted per head to (64 reals | 64 imags); rotation = x*cc +
swap64(x)*ss with host-precomputed cc/ss tables (swap on ACT, mul on
Pool, mul+add on DVE).

Causality at element granularity: key blocks above the diagonal are
skipped, diagonal-strip blocks compute S/exp/PV only on their valid
column range, and the single triangular 128x128 self-block gets a mask
multiply. Softmax denominator: es blocks accumulate into two partial
sums (even kc on DVE, odd kc on Pool — independent chains), combined +
partition-summed by one ones-matmul per (h, qt), deferred by one unit.

All matmuls are bf16 (fp8 PV was tested: quantization of softmax
weights/v does NOT average down against zero-mean v — 2.6e-2 rel err,
over the 2e-2 gate). Weights/x are pre-arranged on the host so every
SBUF load is contiguous per partition; x is token-chunked across two
DMA queues so the v projection starts ~4us in; output stores alternate
sync/gpsimd queues.
"""

import sys

sys.path.insert(0, "/opt/trn_rl_repo")

import numpy as np

import concourse.bacc as bacc
import concourse.mybir as mybir
import concourse.tile as tile
from concourse.bass_utils import run_bass_kernel_spmd

F32 = mybir.dt.float32
BF16 = mybir.dt.bfloat16
FP8 = mybir.dt.float8e4
MM_DT = BF16
N_CORES = 8
QT = 512


def build_nc(B, T, C, repeat=1):
    D = 128
    Hc = (C // D) * B // N_CORES   # 4 heads per core
    ncc = C // 128
    nkc = T // 128
    nqt = T // QT
    ndg = QT // 128

    nc = bacc.Bacc(None)
    xT = nc.dram_tensor("xT", [128, ncc, T], MM_DT, kind="ExternalInput")
    Wq = nc.dram_tensor("Wq", [128, Hc, ncc, 128], MM_DT, kind="ExternalInput")
    Wk = nc.dram_tensor("Wk", [128, Hc, ncc, 128], MM_DT, kind="ExternalInput")
    Wv = nc.dram_tensor("Wv", [128, ncc, Hc * 128], MM_DT, kind="ExternalInput")
    Wp = nc.dram_tensor("Wp", [128, Hc, ncc, 128], MM_DT, kind="ExternalInput")
    cc = nc.dram_tensor("cc", [128, T], BF16, kind="ExternalInput")
    ss = nc.dram_tensor("ss", [128, T], BF16, kind="ExternalInput")
    tri = nc.dram_tensor("tri", [128, 128], MM_DT, kind="ExternalInput")
    ones = nc.dram_tensor("ones", [128, 128], MM_DT, kind="ExternalInput")
    po = nc.dram_tensor("po", [C, T], F32, kind="ExternalOutput")

    scale = 1.0 / float(np.sqrt(np.float32(D)))

    with tile.TileContext(nc) as tc:
      from contextlib import ExitStack
      with ExitStack() as tp:
        p_yv = tp.enter_context(tc.tile_pool(name="yv", bufs=1))
        p_qk = tp.enter_context(tc.tile_pool(name="qk", bufs=2))
        # PSUM budget (8 banks): proj/phC 3 + S 2 + y 2 + cs 1 = 8
        p_ps = tp.enter_context(tc.tile_pool(name="ps", bufs=3, space="PSUM"))
        p_S = tp.enter_context(tc.tile_pool(name="S", bufs=2, space="PSUM"))
        p_yps = tp.enter_context(tc.tile_pool(name="yps", bufs=2, space="PSUM"))
        p_cs = tp.enter_context(tc.tile_pool(name="cs", bufs=1, space="PSUM"))
        for _rep in range(repeat):
          y_sb = p_yv.tile([128, Hc, T], MM_DT, tag="y")
          v_sb = p_yv.tile([128, nkc, Hc * 128], MM_DT, tag="v")

          with (
              tc.tile_pool(name="tabs", bufs=1) as p_tabs,
              tc.tile_pool(name="wstr", bufs=2) as p_w,
              tc.tile_pool(name="wp", bufs=1) as p_wp,
              tc.tile_pool(name="es", bufs=nkc + 1) as p_es,
              tc.tile_pool(name="dn", bufs=2) as p_dn,
          ):
            wp_sb = p_wp.tile([128, Hc, ncc, 128], MM_DT, tag="wp")
            cc_sb = p_tabs.tile([128, T], BF16, tag="cc")
            ss_sb = p_tabs.tile([128, T], BF16, tag="ss")
            tri_sb = p_tabs.tile([128, 128], MM_DT, tag="tri")
            ones_sb = p_tabs.tile([128, 128], MM_DT, tag="ones")

            state = {"pend": None}

            def flush_pend():
                h_, qsl_, yps_, esA_, esB_, bc0 = state["pend"]
                fin = p_dn.tile([128, QT], MM_DT, tag="fin")
                if bc0 > 0:
                    nc.vector.tensor_copy(fin[:, 0:bc0], esA_[:, 0:bc0])
                nc.vector.tensor_add(
                    fin[:, bc0:QT], esA_[:, bc0:QT], esB_[:, bc0:QT])
                csps = p_cs.tile([128, QT], F32, tag="cs")
                nc.tensor.matmul(csps[:], ones_sb[:], fin[:],
                                 start=True, stop=True)
                rc = p_dn.tile([128, QT], F32, tag="rc")
                nc.vector.reciprocal(rc[:], csps[:])
                nc.vector.tensor_mul(y_sb[:, h_, qsl_], yps_[:], rc[:])

            def attn_head(h, qh, kh, fill_stream, rate):
                """Causal attention for head h; drains fill_stream between
                blocks at `rate` items per block."""
                credit = [0.0]

                def drain(n):
                    if fill_stream is None:
                        return
                    credit[0] += n
                    while credit[0] >= 1.0:
                        fn = next(fill_stream, None)
                        if fn is None:
                            credit[0] = 0.0
                            return
                        fn()
                        credit[0] -= 1.0

                for qt in range(nqt):
                    nkv = ndg * (qt + 1)
                    qsl = slice(qt * QT, (qt + 1) * QT)
                    esA = p_dn.tile([128, QT], F32, tag="esA")
                    esB = p_dn.tile([128, QT], MM_DT, tag="esB")
                    esB_c0 = None  # first valid col of esB
                    es_list = []
                    for kc in range(nkv):
                        i = kc - ndg * qt
                        c0 = max(0, i) * 128  # valid cols [c0:QT)
                        sps = p_S.tile([128, QT], F32, tag="S")
                        nc.tensor.matmul(
                            sps[:, c0:QT], kh[:, kc * 128:(kc + 1) * 128],
                            qh[:, qt * QT + c0:(qt + 1) * QT],
                            start=True, stop=True,
                        )
                        es = p_es.tile([128, QT], MM_DT, tag="es")
                        nc.scalar.activation(
                            es[:, c0:QT], sps[:, c0:QT],
                            mybir.ActivationFunctionType.Exp, scale=scale,
                        )
                        if i >= 0:  # triangular self-block
                            nc.vector.tensor_mul(
                                es[:, c0:c0 + 128], es[:, c0:c0 + 128],
                                tri_sb[:],
                            )
                        if kc == 0:
                            nc.vector.tensor_copy(esA[:], es[:])
                        elif kc % 2 == 0:
                            nc.vector.tensor_add(
                                esA[:, c0:QT], esA[:, c0:QT], es[:, c0:QT])
                        elif esB_c0 is None:
                            esB_c0 = c0
                            nc.gpsimd.tensor_copy(esB[:, c0:QT], es[:, c0:QT])
                        else:
                            nc.gpsimd.tensor_add(
                                esB[:, c0:QT], esB[:, c0:QT], es[:, c0:QT])
                        es_list.append((es, c0))
                        drain(rate)
                    if state["pend"] is not None:
                        flush_pend()
                    yps = p_yps.tile([128, QT], F32, tag="y")
                    for kc, (es, c0) in enumerate(es_list):
                        nc.tensor.matmul(
                            yps[:, c0:QT],
                            v_sb[:, kc, h * 128:(h + 1) * 128],
                            es[:, c0:QT],
                            start=(kc == 0), stop=(kc == nkv - 1),
                        )
                        drain(rate)
                    state["pend"] = (h, qsl, yps, esA, esB, esB_c0 or 0)

            # ---------- phase C emission, as a drainable stream ----------
            def phc_stream(p_ost):
                gidx = 0
                for qtp in range(nqt // 2):
                    for oc in range(ncc):
                        ops = [p_ps.tile([128, QT], F32, tag="mm",
                                         name=f"opc{gidx}_{j}")
                               for j in range(2)]

                        def grp(ops=ops, oc=oc, qtp=qtp, gidx=gidx):
                            for hh in range(Hc):
                                for j in range(2):
                                    qt = qtp * 2 + j
                                    nc.tensor.matmul(
                                        ops[j][:], wp_sb[:, hh, oc, :],
                                        y_sb[:, hh, qt * QT:(qt + 1) * QT],
                                        start=(hh == 0), stop=(hh == Hc - 1),
                                    )
                            for j in range(2):
                                qt = qtp * 2 + j
                                ost = p_ost.tile([128, QT], F32, tag="ost")
                                if gidx % 2 == 0:
                                    nc.vector.tensor_copy(ost[:], ops[j][:])
                                else:
                                    nc.scalar.copy(ost[:], ops[j][:])
                                seng = (nc.sync if (gidx + j) % 2 == 0
                                        else nc.gpsimd)
                                seng.dma_start(
                                    po[oc * 128:(oc + 1) * 128,
                                       qt * QT:(qt + 1) * QT], ost[:])
                        yield grp
                        gidx += 1

            w_tiles = {}
            qk_tiles = {}

            def fetch_w(hh):
                if hh >= Hc:
                    return
                wqh = p_w.tile([128, ncc, 128], MM_DT, tag="wqh")
                nc.sync.dma_start(wqh[:], Wq[:, hh])
                wkh = p_w.tile([128, ncc, 128], MM_DT, tag="wkh")
                nc.sync.dma_start(wkh[:], Wk[:, hh])
                w_tiles[hh] = (wqh, wkh)

            with (
                tc.tile_pool(name="xw", bufs=1) as p_xw,
                tc.tile_pool(name="rope", bufs=2) as p_rope,
            ):
                # ---- DMAs: x token-chunked so v-proj starts right away ----
                wv_sb = p_xw.tile([128, ncc, Hc * 128], MM_DT, tag="wv")
                qv = ncc // 4
                for i in range(4):
                    nc.scalar.dma_start(wv_sb[:, i * qv:(i + 1) * qv, :],
                                        Wv[:, i * qv:(i + 1) * qv, :])
                x_sb = p_xw.tile([128, ncc, T], MM_DT, tag="x")
                ntk = 8
                tkw = T // ntk
                for tk in range(ntk):
                    eng = nc.sync if tk % 2 == 0 else nc.scalar
                    eng.dma_start(
                        x_sb[:, :, tk * tkw:(tk + 1) * tkw],
                        xT[:, :, tk * tkw:(tk + 1) * tkw])
                nc.sync.dma_start(cc_sb[:], cc[:])
                nc.sync.dma_start(ss_sb[:], ss[:])
                nc.sync.dma_start(tri_sb[:], tri[:])
                nc.sync.dma_start(ones_sb[:], ones[:])

                # ---- v projection (tokens on partitions), block pairs ----
                for pr in range(nkc // 2):
                    vps = [p_ps.tile([128, Hc * 128], F32, tag="mm",
                                     name=f"vps{j}") for j in range(2)]
                    for c in range(ncc):
                        for j in range(2):
                            tb = pr * 2 + j
                            nc.tensor.matmul(
                                vps[j][:], x_sb[:, c, tb * 128:(tb + 1) * 128],
                                wv_sb[:, c, :],
                                start=(c == 0), stop=(c == ncc - 1),
                            )
                    for j in range(2):
                        nc.vector.tensor_copy(v_sb[:, pr * 2 + j, :], vps[j][:])

                def rope(dst_ap, src_ps, cc_t, ss_t):
                    """dst = src*cc + swap64(src)*ss; src stays in PSUM."""
                    sw = p_rope.tile([128, QT], BF16, tag="rp_sw")
                    nc.scalar.copy(sw[0:64, :], src_ps[64:128, :])
                    nc.scalar.copy(sw[64:128, :], src_ps[0:64, :])
                    nc.gpsimd.tensor_mul(sw[:], sw[:], ss_t)
                    nc.vector.tensor_mul(dst_ap, src_ps[:], cc_t)
                    nc.vector.tensor_add(dst_ap, dst_ap, sw[:])

                def proj_stream(hh):
                    """Yield emit-callbacks for head hh's k/q projection."""
                    if hh >= Hc:
                        return
                    qh = p_qk.tile([128, T], MM_DT, tag="qh", name=f"qh{hh}")
                    kh = p_qk.tile([128, T], MM_DT, tag="kh", name=f"kh{hh}")
                    qk_tiles[hh] = (qh, kh)
                    wqh, wkh = w_tiles.pop(hh)
                    for w_sb, dst in ((wkh, kh), (wqh, qh)):
                        for pr in range(nqt // 2):
                            ps = [p_ps.tile([128, QT], F32, tag="mm",
                                            name=f"ps{hh}_{j}")
                                  for j in range(2)]
                            for c in range(ncc):
                                for j in range(2):
                                    tt = pr * 2 + j

                                    def mm(w_sb=w_sb, ps_t=ps[j], c=c, tt=tt):
                                        nc.tensor.matmul(
                                            ps_t[:], w_sb[:, c, :],
                                            x_sb[:, c,
                                                 tt * QT:(tt + 1) * QT],
                                            start=(c == 0),
                                            stop=(c == ncc - 1),
                                        )
                                    yield mm
                            for j in range(2):
                                tt = pr * 2 + j
                                sl = slice(tt * QT, (tt + 1) * QT)

                                def rp(dst=dst, ps_t=ps[j], sl=sl):
                                    rope(dst[:, sl], ps_t,
                                         cc_sb[:, sl], ss_sb[:, sl])
                                yield rp

                fetch_w(0)
                fetch_w(1)
                for fn in proj_stream(0):
                    fn()

                for h in range(Hc - 1):
                    fetch_w(h + 2)
                    pstream = proj_stream(h + 1)
                    qh, kh = qk_tiles[h]
                    attn_head(h, qh, kh, pstream, 1.6)
                    for fn in pstream:  # leftover proj of h+1
                        fn()

            # x/wv/rope freed; last head's attention drains phase C qtp0
            with tc.tile_pool(name="ost", bufs=6) as p_ost:
                for chk in range(4):
                    nc.sync.dma_start(
                        wp_sb[:, :, chk * 4:(chk + 1) * 4, :],
                        Wp[:, :, chk * 4:(chk + 1) * 4, :])
                phcs = phc_stream(p_ost)
                h = Hc - 1
                qh, kh = qk_tiles[h]
                # Only the ncc qtp0 groups are dep-safe during the last
                # head's attention (they read y[*, qt0/qt1] only), and only
                # once the (h3, qt1) flush has been EMITTED — i.e. from
                # qt2's PV phase onward. Anything more would enqueue a PE
                # matmul that waits on a flush emitted later in the same
                # in-order queue (deadlock).
                credit = [0.0]
                emitted = [0]

                def fill(n, qt, pv):
                    if not (qt > 2 or (qt == 2 and pv)):
                        return
                    if emitted[0] >= ncc:
                        return
                    credit[0] += n
                    while credit[0] >= 1.0 and emitted[0] < ncc:
                        fn = next(phcs, None)
                        if fn is None:
                            credit[0] = 0.0
                            return
                        fn()
                        emitted[0] += 1
                        credit[0] -= 1.0

                # inline attention for the last head with phC filler
                for qt in range(nqt):
                    nkv = ndg * (qt + 1)
                    qsl = slice(qt * QT, (qt + 1) * QT)
                    esA = p_dn.tile([128, QT], F32, tag="esA")
                    esB = p_dn.tile([128, QT], MM_DT, tag="esB")
                    esB_c0 = None
                    es_list = []
                    for kc in range(nkv):
                        i = kc - ndg * qt
                        c0 = max(0, i) * 128
                        sps = p_S.tile([128, QT], F32, tag="S")
                        nc.tensor.matmul(
                            sps[:, c0:QT], kh[:, kc * 128:(kc + 1) * 128],
                            qh[:, qt * QT + c0:(qt + 1) * QT],
                            start=True, stop=True,
                        )
                        es = p_es.tile([128, QT], MM_DT, tag="es")
                        nc.scalar.activation(
                            es[:, c0:QT], sps[:, c0:QT],
                            mybir.ActivationFunctionType.Exp, scale=scale,
                        )
                        if i >= 0:
                            nc.vector.tensor_mul(
                                es[:, c0:c0 + 128], es[:, c0:c0 + 128],
                                tri_sb[:],
                            )
                        if kc == 0:
                            nc.vector.tensor_copy(esA[:], es[:])
                        elif kc % 2 == 0:
                            nc.vector.tensor_add(
                                esA[:, c0:QT], esA[:, c0:QT], es[:, c0:QT])
                        elif esB_c0 is None:
                            esB_c0 = c0
                            nc.gpsimd.tensor_copy(esB[:, c0:QT], es[:, c0:QT])
                        else:
                            nc.gpsimd.tensor_add(
                                esB[:, c0:QT], esB[:, c0:QT], es[:, c0:QT])
                        es_list.append((es, c0))
                        fill(0.5, qt, False)
                    if state["pend"] is not None:
                        flush_pend()
                    yps = p_yps.tile([128, QT], F32, tag="y")
                    for kc, (es, c0) in enumerate(es_list):
                        nc.tensor.matmul(
                            yps[:, c0:QT],
                            v_sb[:, kc, h * 128:(h + 1) * 128],
                            es[:, c0:QT],
                            start=(kc == 0), stop=(kc == nkv - 1),
                        )
                        fill(0.5, qt, True)
                    state["pend"] = (h, qsl, yps, esA, esB, esB_c0 or 0)
                flush_pend()
                state["pend"] = None

                # ---- rest of the partial output projection ----
                for fn in phcs:
                    fn()

    nc.compile()
    return nc


def _prep_inputs(x, W_attn, W_proj, rope_cos, rope_sin, B, T, C):
    import ml_dtypes
    mmnp = ml_dtypes.bfloat16
    D = 128
    H = C // D
    Hc = H * B // N_CORES
    ncc = C // 128

    perm = np.concatenate([np.arange(0, D, 2), np.arange(1, D, 2)])
    cosT = rope_cos.T.astype(np.float32)
    sinT = rope_sin.T.astype(np.float32)
    cc = np.concatenate([cosT, cosT], axis=0).astype(mmnp)
    ss = np.concatenate([-sinT, sinT], axis=0).astype(mmnp)

    tri = (np.arange(128)[:, None] <= np.arange(128)[None, :]).astype(mmnp)
    trib = np.where(np.arange(128)[:, None] <= np.arange(128)[None, :],
                    0.0, -1e30).astype(np.float32)
    ones = np.ones((128, 128), dtype=mmnp)

    xTs = [
        np.ascontiguousarray(
            x[b].T.reshape(ncc, 128, T).transpose(1, 0, 2)).astype(mmnp)
        for b in range(B)
    ]

    def stat_tiles(w):  # [C, Hc*128] -> [128, Hc, ncc, 128]
        return np.ascontiguousarray(
            w.reshape(ncc, 128, Hc, 128).transpose(1, 2, 0, 3)).astype(mmnp)

    groups = []
    for g in range(N_CORES // B):
        hsl = np.arange(g * Hc * D, (g + 1) * Hc * D)
        cperm = np.concatenate([g * Hc * D + h * D + perm for h in range(Hc)])
        Wq_t = stat_tiles(W_attn[:, 0:C][:, cperm])
        Wk_t = stat_tiles(W_attn[:, C:2 * C][:, cperm])
        Wv_t = np.ascontiguousarray(
            W_attn[:, 2 * C:3 * C][:, hsl]
            .reshape(ncc, 128, Hc * 128).transpose(1, 0, 2)).astype(mmnp)
        Wp_t = np.ascontiguousarray(
            W_proj[hsl, :].reshape(Hc, 128, ncc, 128)
            .transpose(1, 0, 2, 3)).astype(mmnp)
        groups.append((Wq_t, Wk_t, Wv_t, Wp_t))

    in_maps = []
    for m in range(N_CORES):
        b = m // (N_CORES // B)
        Wq_t, Wk_t, Wv_t, Wp_t = groups[m % (N_CORES // B)]
        in_maps.append({
            "xT": xTs[b], "Wq": Wq_t, "Wk": Wk_t, "Wv": Wv_t, "Wp": Wp_t,
            "cc": cc, "ss": ss, "tri": tri, "trib": trib, "ones": ones,
        })
    return in_maps


_NC_CACHE = {}


def run(x, W_attn, W_proj, rope_cos, rope_sin, attention_mask=None, trace=False):
    B, T, C = x.shape
    key = (B, T, C)
    if key not in _NC_CACHE:
        _NC_CACHE[key] = build_nc(B, T, C)
    nc = _NC_CACHE[key]
    in_maps = _prep_inputs(
        np.asarray(x, dtype=np.float32),
        np.asarray(W_attn, dtype=np.float32),
        np.asarray(W_proj, dtype=np.float32),
        np.asarray(rope_cos, dtype=np.float32),
        np.asarray(rope_sin, dtype=np.float32),
        B, T, C,
    )
    res = run_bass_kernel_spmd(nc, in_maps, list(range(N_CORES)), trace=trace)
    gpb = N_CORES // B
    out = np.empty((B, T, C), dtype=np.float32)
    for b in range(B):
        acc = res.results[b * gpb]["po"].astype(np.float64)
        for j in range(1, gpb):
            acc += res.results[b * gpb + j]["po"]
        out[b] = acc.T
    return out, res


def kernel(x, W_attn, W_proj, rope_cos, rope_sin, attention_mask):
    out, _ = run(x, W_attn, W_proj, rope_cos, rope_sin)
    return out


# revision 5
# speedup vs baseline: 1.1981x; 1.1981x over previous
"""Causal self-attention (RoPE) TRN2 kernel — head-tensor-parallel, fused.

Sharding (per spec hint): core m = (batch m//4, heads 4*(m%4)..+4). Each
core projects q/k/v for its 4 heads over the full sequence, runs causal
attention, and applies its row-slice of W_proj, producing a PARTIAL
output [C, T] f32. The host sums the 4 partials per batch (the
post-c_proj all-reduce) and transposes. No on-device collectives.

One fused instruction stream per core, software-pipelined so the PE
(tensor) engine never waits on the ACT exp chain or the DVE/Pool
softmax work:

  [v proj] [k/q proj h0] [attn h0 || k/q proj h1] [attn h1 || proj h2]
  [attn h2 || proj h3] [attn h3 || out-proj qtp0] [out-proj rest]

Head h's attention blocks are interleaved with head h+1's projection
matmuls (~1.6 proj MMs per attention block); the last head's attention
interleaves the first half of the output projection instead (gated on
the already-flushed y tiles; capped so no emitted matmul can wait on a
flush that is emitted later in the in-order PE queue).

On-chip layout is fully transposed (channels on partitions, tokens on
the free dim): qT/kT = [D, T] per head, v = [tokens, D], S^T = [keys, q]
so the PV matmul needs no transposes. RoPE: W_attn q/k columns are
permuted per head to (64 reals | 64 imags); rotation = x*cc +
swap64(x)*ss with host-precomputed cc/ss tables (swap on ACT, mul on
Pool, mul+add on DVE).

Causality at element granularity: key blocks above the diagonal are
skipped, diagonal-strip blocks compute S/exp/PV only on their valid
column range, and the single triangular 128x128 self-block gets a mask
multiply. Softmax denominator: es blocks accumulate into two partial
sums (even kc on DVE, odd kc on Pool — independent chains), combined +
partition-summed by one ones-matmul per (h, qt), deferred by one unit.

All matmuls are bf16 (fp8 PV was tested: quantization of softmax
weights/v does NOT average down against zero-mean v — 2.6e-2 rel err,
over the 2e-2 gate). Weights/x are pre-arranged on the host so every
SBUF load is contiguous per partition; x is token-chunked across two
DMA queues so the v projection starts ~4us in; output stores alternate
sync/gpsimd queues.
"""

import sys

sys.path.insert(0, "/opt/trn_rl_repo")

import numpy as np

import concourse.bacc as bacc
import concourse.mybir as mybir
import concourse.tile as tile
from concourse.bass_utils import run_bass_kernel_spmd

F32 = mybir.dt.float32
BF16 = mybir.dt.bfloat16
FP8 = mybir.dt.float8e4
MM_DT = BF16
N_CORES = 8
QT = 512


def build_nc(B, T, C, repeat=1, rate=3.0):
    D = 128
    Hc = (C // D) * B // N_CORES   # 4 heads per core
    ncc = C // 128
    nkc = T // 128
    nqt = T // QT
    ndg = QT // 128

    nc = bacc.Bacc(None)
    xT = nc.dram_tensor("xT", [128, ncc, T], MM_DT, kind="ExternalInput")
    Wq = nc.dram_tensor("Wq", [128, Hc, ncc, 128], MM_DT, kind="ExternalInput")
    Wk = nc.dram_tensor("Wk", [128, Hc, ncc, 128], MM_DT, kind="ExternalInput")
    Wv = nc.dram_tensor("Wv", [128, ncc, Hc * 128], MM_DT, kind="ExternalInput")
    Wp = nc.dram_tensor("Wp", [128, Hc, ncc, 128], MM_DT, kind="ExternalInput")
    cc = nc.dram_tensor("cc", [128, T], BF16, kind="ExternalInput")
    ss = nc.dram_tensor("ss", [128, T], BF16, kind="ExternalInput")
    tri = nc.dram_tensor("tri", [128, 128], MM_DT, kind="ExternalInput")
    ones = nc.dram_tensor("ones", [128, 128], MM_DT, kind="ExternalInput")
    po = nc.dram_tensor("po", [C, T], F32, kind="ExternalOutput")

    scale = 1.0 / float(np.sqrt(np.float32(D)))

    with tile.TileContext(nc) as tc:
      from contextlib import ExitStack
      with ExitStack() as tp:
        p_yv = tp.enter_context(tc.tile_pool(name="yv", bufs=1))
        p_qk = tp.enter_context(tc.tile_pool(name="qk", bufs=2))
        # PSUM budget (8 banks): proj/phC 3 + S 2 + y 2 + cs 1 = 8
        p_ps = tp.enter_context(tc.tile_pool(name="ps", bufs=3, space="PSUM"))
        p_S = tp.enter_context(tc.tile_pool(name="S", bufs=2, space="PSUM"))
        p_yps = tp.enter_context(tc.tile_pool(name="yps", bufs=2, space="PSUM"))
        p_cs = tp.enter_context(tc.tile_pool(name="cs", bufs=1, space="PSUM"))
        for _rep in range(repeat):
          y_sb = p_yv.tile([128, Hc, T], MM_DT, tag="y")
          v_sb = p_yv.tile([128, nkc, Hc * 128], MM_DT, tag="v")

          with (
              tc.tile_pool(name="tabs", bufs=1) as p_tabs,
              tc.tile_pool(name="wstr", bufs=2) as p_w,
              tc.tile_pool(name="wp", bufs=1) as p_wp,
              tc.tile_pool(name="es", bufs=nkc + 1) as p_es,
              tc.tile_pool(name="dn", bufs=2) as p_dn,
          ):
            wp_sb = p_wp.tile([128, Hc, ncc, 128], MM_DT, tag="wp")
            cc_sb = p_tabs.tile([128, T], BF16, tag="cc")
            ss_sb = p_tabs.tile([128, T], BF16, tag="ss")
            tri_sb = p_tabs.tile([128, 128], MM_DT, tag="tri")
            ones_sb = p_tabs.tile([128, 128], MM_DT, tag="ones")

            state = {"pend": None}

            def flush_pend():
                h_, qsl_, yps_, esA_, esB_, bc0 = state["pend"]
                fin = p_dn.tile([128, QT], MM_DT, tag="fin")
                if bc0 > 0:
                    nc.vector.tensor_copy(fin[:, 0:bc0], esA_[:, 0:bc0])
                nc.vector.tensor_add(
                    fin[:, bc0:QT], esA_[:, bc0:QT], esB_[:, bc0:QT])
                csps = p_cs.tile([128, QT], F32, tag="cs")
                nc.tensor.matmul(csps[:], ones_sb[:], fin[:],
                                 start=True, stop=True)
                rc = p_dn.tile([128, QT], F32, tag="rc")
                nc.vector.reciprocal(rc[:], csps[:])
                nc.vector.tensor_mul(y_sb[:, h_, qsl_], yps_[:], rc[:])

            def attn_head(h, qh, kh, fill_stream, rate):
                """Causal attention for head h; drains fill_stream between
                blocks at `rate` items per block."""
                credit = [0.0]

                def drain(n):
                    if fill_stream is None:
                        return
                    credit[0] += n
                    while credit[0] >= 1.0:
                        fn = next(fill_stream, None)
                        if fn is None:
                            credit[0] = 0.0
                            return
                        fn()
                        credit[0] -= 1.0

                for qt in range(nqt):
                    nkv = ndg * (qt + 1)
                    qsl = slice(qt * QT, (qt + 1) * QT)
                    esA = p_dn.tile([128, QT], F32, tag="esA")
                    esB = p_dn.tile([128, QT], MM_DT, tag="esB")
                    esB_c0 = None  # first valid col of esB
                    es_list = []
                    for kc in range(nkv):
                        i = kc - ndg * qt
                        c0 = max(0, i) * 128  # valid cols [c0:QT)
                        sps = p_S.tile([128, QT], F32, tag="S")
                        nc.tensor.matmul(
                            sps[:, c0:QT], kh[:, kc * 128:(kc + 1) * 128],
                            qh[:, qt * QT + c0:(qt + 1) * QT],
                            start=True, stop=True,
                        )
                        es = p_es.tile([128, QT], MM_DT, tag="es")
                        nc.scalar.activation(
                            es[:, c0:QT], sps[:, c0:QT],
                            mybir.ActivationFunctionType.Exp, scale=scale,
                        )
                        if i >= 0:  # triangular self-block
                            nc.vector.tensor_mul(
                                es[:, c0:c0 + 128], es[:, c0:c0 + 128],
                                tri_sb[:],
                            )
                        if kc == 0:
                            nc.vector.tensor_copy(esA[:], es[:])
                        elif kc % 2 == 0:
                            nc.vector.tensor_add(
                                esA[:, c0:QT], esA[:, c0:QT], es[:, c0:QT])
                        elif esB_c0 is None:
                            esB_c0 = c0
                            nc.gpsimd.tensor_copy(esB[:, c0:QT], es[:, c0:QT])
                        else:
                            nc.gpsimd.tensor_add(
                                esB[:, c0:QT], esB[:, c0:QT], es[:, c0:QT])
                        es_list.append((es, c0))
                        drain(rate)
                    if state["pend"] is not None:
                        flush_pend()
                    yps = p_yps.tile([128, QT], F32, tag="y")
                    for kc, (es, c0) in enumerate(es_list):
                        nc.tensor.matmul(
                            yps[:, c0:QT],
                            v_sb[:, kc, h * 128:(h + 1) * 128],
                            es[:, c0:QT],
                            start=(kc == 0), stop=(kc == nkv - 1),
                        )
                        drain(rate)
                    state["pend"] = (h, qsl, yps, esA, esB, esB_c0 or 0)

            # ---------- phase C emission, as a drainable stream ----------
            def phc_stream(p_ost):
                gidx = 0
                for qtp in range(nqt // 2):
                    for oc in range(ncc):
                        ops = [p_ps.tile([128, QT], F32, tag="mm",
                                         name=f"opc{gidx}_{j}")
                               for j in range(2)]

                        def grp(ops=ops, oc=oc, qtp=qtp, gidx=gidx):
                            for hh in range(Hc):
                                for j in range(2):
                                    qt = qtp * 2 + j
                                    nc.tensor.matmul(
                                        ops[j][:], wp_sb[:, hh, oc, :],
                                        y_sb[:, hh, qt * QT:(qt + 1) * QT],
                                        start=(hh == 0), stop=(hh == Hc - 1),
                                    )
                            for j in range(2):
                                qt = qtp * 2 + j
                                ost = p_ost.tile([128, QT], F32, tag="ost")
                                if gidx % 2 == 0:
                                    nc.vector.tensor_copy(ost[:], ops[j][:])
                                else:
                                    nc.scalar.copy(ost[:], ops[j][:])
                                seng = (nc.sync if (gidx + j) % 2 == 0
                                        else nc.gpsimd)
                                seng.dma_start(
                                    po[oc * 128:(oc + 1) * 128,
                                       qt * QT:(qt + 1) * QT], ost[:])
                        yield grp
                        gidx += 1

            w_tiles = {}
            qk_tiles = {}

            def fetch_w(hh):
                if hh >= Hc:
                    return
                wqh = p_w.tile([128, ncc, 128], MM_DT, tag="wqh")
                nc.sync.dma_start(wqh[:], Wq[:, hh])
                wkh = p_w.tile([128, ncc, 128], MM_DT, tag="wkh")
                nc.sync.dma_start(wkh[:], Wk[:, hh])
                w_tiles[hh] = (wqh, wkh)

            with (
                tc.tile_pool(name="xw", bufs=1) as p_xw,
                tc.tile_pool(name="rope", bufs=2) as p_rope,
            ):
                # ---- DMAs: x token-chunked so v-proj starts right away ----
                wv_sb = p_xw.tile([128, ncc, Hc * 128], MM_DT, tag="wv")
                qv = ncc // 4
                for i in range(4):
                    nc.scalar.dma_start(wv_sb[:, i * qv:(i + 1) * qv, :],
                                        Wv[:, i * qv:(i + 1) * qv, :])
                x_sb = p_xw.tile([128, ncc, T], MM_DT, tag="x")
                ntk = 8
                tkw = T // ntk
                for tk in range(ntk):
                    eng = nc.sync if tk % 2 == 0 else nc.scalar
                    eng.dma_start(
                        x_sb[:, :, tk * tkw:(tk + 1) * tkw],
                        xT[:, :, tk * tkw:(tk + 1) * tkw])
                nc.sync.dma_start(cc_sb[:], cc[:])
                nc.sync.dma_start(ss_sb[:], ss[:])
                nc.sync.dma_start(tri_sb[:], tri[:])
                nc.sync.dma_start(ones_sb[:], ones[:])

                # ---- v projection (tokens on partitions), block pairs ----
                for pr in range(nkc // 2):
                    vps = [p_ps.tile([128, Hc * 128], F32, tag="mm",
                                     name=f"vps{j}") for j in range(2)]
                    for c in range(ncc):
                        for j in range(2):
                            tb = pr * 2 + j
                            nc.tensor.matmul(
                                vps[j][:], x_sb[:, c, tb * 128:(tb + 1) * 128],
                                wv_sb[:, c, :],
                                start=(c == 0), stop=(c == ncc - 1),
                            )
                    for j in range(2):
                        nc.vector.tensor_copy(v_sb[:, pr * 2 + j, :], vps[j][:])

                def rope(dst_ap, src_ps, cc_t, ss_t):
                    """dst = src*cc + swap64(src)*ss; src stays in PSUM."""
                    sw = p_rope.tile([128, QT], BF16, tag="rp_sw")
                    nc.scalar.copy(sw[0:64, :], src_ps[64:128, :])
                    nc.scalar.copy(sw[64:128, :], src_ps[0:64, :])
                    nc.gpsimd.tensor_mul(sw[:], sw[:], ss_t)
                    nc.vector.tensor_mul(dst_ap, src_ps[:], cc_t)
                    nc.vector.tensor_add(dst_ap, dst_ap, sw[:])

                def proj_stream(hh):
                    """Yield emit-callbacks for head hh's k/q projection."""
                    if hh >= Hc:
                        return
                    qh = p_qk.tile([128, T], MM_DT, tag="qh", name=f"qh{hh}")
                    kh = p_qk.tile([128, T], MM_DT, tag="kh", name=f"kh{hh}")
                    qk_tiles[hh] = (qh, kh)
                    wqh, wkh = w_tiles.pop(hh)
                    for w_sb, dst in ((wkh, kh), (wqh, qh)):
                        for pr in range(nqt // 2):
                            ps = [p_ps.tile([128, QT], F32, tag="mm",
                                            name=f"ps{hh}_{j}")
                                  for j in range(2)]
                            for c in range(ncc):
                                for j in range(2):
                                    tt = pr * 2 + j

                                    def mm(w_sb=w_sb, ps_t=ps[j], c=c, tt=tt):
                                        nc.tensor.matmul(
                                            ps_t[:], w_sb[:, c, :],
                                            x_sb[:, c,
                                                 tt * QT:(tt + 1) * QT],
                                            start=(c == 0),
                                            stop=(c == ncc - 1),
                                        )
                                    yield mm
                            for j in range(2):
                                tt = pr * 2 + j
                                sl = slice(tt * QT, (tt + 1) * QT)

                                def rp(dst=dst, ps_t=ps[j], sl=sl):
                                    rope(dst[:, sl], ps_t,
                                         cc_sb[:, sl], ss_sb[:, sl])
                                yield rp

                fetch_w(0)
                fetch_w(1)
                for fn in proj_stream(0):
                    fn()

                for h in range(Hc - 1):
                    fetch_w(h + 2)
                    pstream = proj_stream(h + 1)
                    qh, kh = qk_tiles[h]
                    attn_head(h, qh, kh, pstream, rate)
                    for fn in pstream:  # leftover proj of h+1
                        fn()

            # x/wv/rope freed; last head's attention drains phase C qtp0
            with tc.tile_pool(name="ost", bufs=6) as p_ost:
                for chk in range(4):
                    nc.sync.dma_start(
                        wp_sb[:, :, chk * 4:(chk + 1) * 4, :],
                        Wp[:, :, chk * 4:(chk + 1) * 4, :])
                phcs = phc_stream(p_ost)
                h = Hc - 1
                qh, kh = qk_tiles[h]
                # Only the ncc qtp0 groups are dep-safe during the last
                # head's attention (they read y[*, qt0/qt1] only), and only
                # once the (h3, qt1) flush has been EMITTED — i.e. from
                # qt2's PV phase onward. Anything more would enqueue a PE
                # matmul that waits on a flush emitted later in the same
                # in-order queue (deadlock).
                credit = [0.0]
                emitted = [0]

                def fill(n, qt, pv):
                    if not (qt > 2 or (qt == 2 and pv)):
                        return
                    if emitted[0] >= ncc:
                        return
                    credit[0] += n
                    while credit[0] >= 1.0 and emitted[0] < ncc:
                        fn = next(phcs, None)
                        if fn is None:
                            credit[0] = 0.0
                            return
                        fn()
                        emitted[0] += 1
                        credit[0] -= 1.0

                # inline attention for the last head with phC filler
                for qt in range(nqt):
                    nkv = ndg * (qt + 1)
                    qsl = slice(qt * QT, (qt + 1) * QT)
                    esA = p_dn.tile([128, QT], F32, tag="esA")
                    esB = p_dn.tile([128, QT], MM_DT, tag="esB")
                    esB_c0 = None
                    es_list = []
                    for kc in range(nkv):
                        i = kc - ndg * qt
                        c0 = max(0, i) * 128
                        sps = p_S.tile([128, QT], F32, tag="S")
                        nc.tensor.matmul(
                            sps[:, c0:QT], kh[:, kc * 128:(kc + 1) * 128],
                            qh[:, qt * QT + c0:(qt + 1) * QT],
                            start=True, stop=True,
                        )
                        es = p_es.tile([128, QT], MM_DT, tag="es")
                        nc.scalar.activation(
                            es[:, c0:QT], sps[:, c0:QT],
                            mybir.ActivationFunctionType.Exp, scale=scale,
                        )
                        if i >= 0:
                            nc.vector.tensor_mul(
                                es[:, c0:c0 + 128], es[:, c0:c0 + 128],
                                tri_sb[:],
                            )
                        if kc == 0:
                            nc.vector.tensor_copy(esA[:], es[:])
                        elif kc % 2 == 0:
                            nc.vector.tensor_add(
                                esA[:, c0:QT], esA[:, c0:QT], es[:, c0:QT])
                        elif esB_c0 is None:
                            esB_c0 = c0
                            nc.gpsimd.tensor_copy(esB[:, c0:QT], es[:, c0:QT])
                        else:
                            nc.gpsimd.tensor_add(
                                esB[:, c0:QT], esB[:, c0:QT], es[:, c0:QT])
                        es_list.append((es, c0))
                        fill(0.5, qt, False)
                    if state["pend"] is not None:
                        flush_pend()
                    yps = p_yps.tile([128, QT], F32, tag="y")
                    for kc, (es, c0) in enumerate(es_list):
                        nc.tensor.matmul(
                            yps[:, c0:QT],
                            v_sb[:, kc, h * 128:(h + 1) * 128],
                            es[:, c0:QT],
                            start=(kc == 0), stop=(kc == nkv - 1),
                        )
                        fill(0.5, qt, True)
                    state["pend"] = (h, qsl, yps, esA, esB, esB_c0 or 0)
                flush_pend()
                state["pend"] = None

                # ---- rest of the partial output projection ----
                for fn in phcs:
                    fn()

    nc.compile()
    return nc


def _prep_inputs(x, W_attn, W_proj, rope_cos, rope_sin, B, T, C):
    import ml_dtypes
    mmnp = ml_dtypes.bfloat16
    D = 128
    H = C // D
    Hc = H * B // N_CORES
    ncc = C // 128

    perm = np.concatenate([np.arange(0, D, 2), np.arange(1, D, 2)])
    cosT = rope_cos.T.astype(np.float32)
    sinT = rope_sin.T.astype(np.float32)
    cc = np.concatenate([cosT, cosT], axis=0).astype(mmnp)
    ss = np.concatenate([-sinT, sinT], axis=0).astype(mmnp)

    tri = (np.arange(128)[:, None] <= np.arange(128)[None, :]).astype(mmnp)
    trib = np.where(np.arange(128)[:, None] <= np.arange(128)[None, :],
                    0.0, -1e30).astype(np.float32)
    ones = np.ones((128, 128), dtype=mmnp)

    xTs = [
        np.ascontiguousarray(
            x[b].T.reshape(ncc, 128, T).transpose(1, 0, 2)).astype(mmnp)
        for b in range(B)
    ]

    def stat_tiles(w):  # [C, Hc*128] -> [128, Hc, ncc, 128]
        return np.ascontiguousarray(
            w.reshape(ncc, 128, Hc, 128).transpose(1, 2, 0, 3)).astype(mmnp)

    groups = []
    for g in range(N_CORES // B):
        hsl = np.arange(g * Hc * D, (g + 1) * Hc * D)
        cperm = np.concatenate([g * Hc * D + h * D + perm for h in range(Hc)])
        Wq_t = stat_tiles(W_attn[:, 0:C][:, cperm])
        Wk_t = stat_tiles(W_attn[:, C:2 * C][:, cperm])
        Wv_t = np.ascontiguousarray(
            W_attn[:, 2 * C:3 * C][:, hsl]
            .reshape(ncc, 128, Hc * 128).transpose(1, 0, 2)).astype(mmnp)
        Wp_t = np.ascontiguousarray(
            W_proj[hsl, :].reshape(Hc, 128, ncc, 128)
            .transpose(1, 0, 2, 3)).astype(mmnp)
        groups.append((Wq_t, Wk_t, Wv_t, Wp_t))

    in_maps = []
    for m in range(N_CORES):
        b = m // (N_CORES // B)
        Wq_t, Wk_t, Wv_t, Wp_t = groups[m % (N_CORES // B)]
        in_maps.append({
            "xT": xTs[b], "Wq": Wq_t, "Wk": Wk_t, "Wv": Wv_t, "Wp": Wp_t,
            "cc": cc, "ss": ss, "tri": tri, "trib": trib, "ones": ones,
        })
    return in_maps


_NC_CACHE = {}


def run(x, W_attn, W_proj, rope_cos, rope_sin, attention_mask=None, trace=False):
    B, T, C = x.shape
    key = (B, T, C)
    if key not in _NC_CACHE:
        _NC_CACHE[key] = build_nc(B, T, C)
    nc = _NC_CACHE[key]
    in_maps = _prep_inputs(
        np.asarray(x, dtype=np.float32),
        np.asarray(W_attn, dtype=np.float32),
        np.asarray(W_proj, dtype=np.float32),
        np.asarray(rope_cos, dtype=np.float32),
        np.asarray(rope_sin, dtype=np.float32),
        B, T, C,
    )
    res = run_bass_kernel_spmd(nc, in_maps, list(range(N_CORES)), trace=trace)
    gpb = N_CORES // B
    out = np.empty((B, T, C), dtype=np.float32)
    for b in range(B):
        acc = res.results[b * gpb]["po"].astype(np.float64)
        for j in range(1, gpb):
            acc += res.results[b * gpb + j]["po"]
        out[b] = acc.T
    return out, res


def kernel(x, W_attn, W_proj, rope_cos, rope_sin, attention_mask):
    out, _ = run(x, W_attn, W_proj, rope_cos, rope_sin)
    return out


# revision 6
# speedup vs baseline: 1.2533x; 1.0461x over previous
"""Causal self-attention (RoPE) TRN2 kernel — head-tensor-parallel, fused.

Sharding (per spec hint): core m = (batch m//4, heads 4*(m%4)..+4). Each
core projects q/k/v for its 4 heads over the full sequence, runs causal
attention, and applies its row-slice of W_proj, producing a PARTIAL
output [C, T] in bf16 (partial-sum rounding is negligible next to the
bf16 matmul error, and halving store bytes relieves DMA-queue
contention: -65 us/rep measured). The host sums the 4 partials per
batch in f64 (the post-c_proj all-reduce) and transposes. No on-device
collectives.

One fused instruction stream per core, software-pipelined so the PE
(tensor) engine never waits on the ACT exp chain or the DVE/Pool
softmax work:

  [v proj] [k/q proj h0] [attn h0 || k/q proj h1] [attn h1 || proj h2]
  [attn h2 || proj h3] [attn h3 || out-proj qtp0] [out-proj rest]

Head h's attention blocks are interleaved with head h+1's projection
matmuls (~1.6 proj MMs per attention block); the last head's attention
interleaves the first half of the output projection instead (gated on
the already-flushed y tiles; capped so no emitted matmul can wait on a
flush that is emitted later in the in-order PE queue).

On-chip layout is fully transposed (channels on partitions, tokens on
the free dim): qT/kT = [D, T] per head, v = [tokens, D], S^T = [keys, q]
so the PV matmul needs no transposes. RoPE: W_attn q/k columns are
permuted per head to (64 reals | 64 imags); rotation = x*cc +
swap64(x)*ss with host-precomputed cc/ss tables (swap on ACT, mul on
Pool, mul+add on DVE).

Causality at element granularity: key blocks above the diagonal are
skipped, diagonal-strip blocks compute S/exp/PV only on their valid
column range, and the single triangular 128x128 self-block gets a mask
multiply. Softmax denominator: es blocks accumulate into two partial
sums (even kc on DVE, odd kc on Pool — independent chains), combined +
partition-summed by one ones-matmul per (h, qt), deferred by one unit.

All matmuls are bf16 (fp8 PV was tested: quantization of softmax
weights/v does NOT average down against zero-mean v — 2.6e-2 rel err,
over the 2e-2 gate). Weights/x are pre-arranged on the host so every
SBUF load is contiguous per partition; x is token-chunked across two
DMA queues so the v projection starts ~4us in; output stores alternate
sync/gpsimd queues.
"""

import sys

sys.path.insert(0, "/opt/trn_rl_repo")

import numpy as np

import concourse.bacc as bacc
import concourse.mybir as mybir
import concourse.tile as tile
from concourse.bass_utils import run_bass_kernel_spmd

F32 = mybir.dt.float32
BF16 = mybir.dt.bfloat16
FP8 = mybir.dt.float8e4
MM_DT = BF16
N_CORES = 8
QT = 512


def build_nc(B, T, C, repeat=1, rate=3.0, po_bf16=True):
    D = 128
    Hc = (C // D) * B // N_CORES   # 4 heads per core
    ncc = C // 128
    nkc = T // 128
    nqt = T // QT
    ndg = QT // 128

    nc = bacc.Bacc(None)
    xT = nc.dram_tensor("xT", [128, ncc, T], MM_DT, kind="ExternalInput")
    Wq = nc.dram_tensor("Wq", [128, Hc, ncc, 128], MM_DT, kind="ExternalInput")
    Wk = nc.dram_tensor("Wk", [128, Hc, ncc, 128], MM_DT, kind="ExternalInput")
    Wv = nc.dram_tensor("Wv", [128, ncc, Hc * 128], MM_DT, kind="ExternalInput")
    Wp = nc.dram_tensor("Wp", [128, Hc, ncc, 128], MM_DT, kind="ExternalInput")
    cc = nc.dram_tensor("cc", [128, T], BF16, kind="ExternalInput")
    ss = nc.dram_tensor("ss", [128, T], BF16, kind="ExternalInput")
    tri = nc.dram_tensor("tri", [128, 128], MM_DT, kind="ExternalInput")
    ones = nc.dram_tensor("ones", [128, 128], MM_DT, kind="ExternalInput")
    po = nc.dram_tensor("po", [C, T], BF16 if po_bf16 else F32,
                        kind="ExternalOutput")

    scale = 1.0 / float(np.sqrt(np.float32(D)))

    with tile.TileContext(nc) as tc:
      from contextlib import ExitStack
      with ExitStack() as tp:
        p_yv = tp.enter_context(tc.tile_pool(name="yv", bufs=1))
        p_qk = tp.enter_context(tc.tile_pool(name="qk", bufs=2))
        # PSUM budget (8 banks): proj/phC 3 + S 2 + y 2 + cs 1 = 8
        p_ps = tp.enter_context(tc.tile_pool(name="ps", bufs=3, space="PSUM"))
        p_S = tp.enter_context(tc.tile_pool(name="S", bufs=2, space="PSUM"))
        p_yps = tp.enter_context(tc.tile_pool(name="yps", bufs=2, space="PSUM"))
        p_cs = tp.enter_context(tc.tile_pool(name="cs", bufs=1, space="PSUM"))
        for _rep in range(repeat):
          y_sb = p_yv.tile([128, Hc, T], MM_DT, tag="y")
          v_sb = p_yv.tile([128, nkc, Hc * 128], MM_DT, tag="v")

          with (
              tc.tile_pool(name="tabs", bufs=1) as p_tabs,
              tc.tile_pool(name="wstr", bufs=2) as p_w,
              tc.tile_pool(name="wp", bufs=1) as p_wp,
              tc.tile_pool(name="es", bufs=nkc + 1) as p_es,
              tc.tile_pool(name="dn", bufs=2) as p_dn,
          ):
            wp_sb = p_wp.tile([128, Hc, ncc, 128], MM_DT, tag="wp")
            cc_sb = p_tabs.tile([128, T], BF16, tag="cc")
            ss_sb = p_tabs.tile([128, T], BF16, tag="ss")
            tri_sb = p_tabs.tile([128, 128], MM_DT, tag="tri")
            ones_sb = p_tabs.tile([128, 128], MM_DT, tag="ones")

            state = {"pend": None}

            def flush_pend():
                h_, qsl_, yps_, esA_, esB_, bc0 = state["pend"]
                fin = p_dn.tile([128, QT], MM_DT, tag="fin")
                if bc0 > 0:
                    nc.vector.tensor_copy(fin[:, 0:bc0], esA_[:, 0:bc0])
                nc.vector.tensor_add(
                    fin[:, bc0:QT], esA_[:, bc0:QT], esB_[:, bc0:QT])
                csps = p_cs.tile([128, QT], F32, tag="cs")
                nc.tensor.matmul(csps[:], ones_sb[:], fin[:],
                                 start=True, stop=True)
                rc = p_dn.tile([128, QT], F32, tag="rc")
                nc.vector.reciprocal(rc[:], csps[:])
                nc.vector.tensor_mul(y_sb[:, h_, qsl_], yps_[:], rc[:])

            def attn_head(h, qh, kh, fill_stream, rate):
                """Causal attention for head h; drains fill_stream between
                blocks at `rate` items per block."""
                credit = [0.0]

                def drain(n):
                    if fill_stream is None:
                        return
                    credit[0] += n
                    while credit[0] >= 1.0:
                        fn = next(fill_stream, None)
                        if fn is None:
                            credit[0] = 0.0
                            return
                        fn()
                        credit[0] -= 1.0

                for qt in range(nqt):
                    nkv = ndg * (qt + 1)
                    qsl = slice(qt * QT, (qt + 1) * QT)
                    esA = p_dn.tile([128, QT], F32, tag="esA")
                    esB = p_dn.tile([128, QT], MM_DT, tag="esB")
                    esB_c0 = None  # first valid col of esB
                    es_list = []
                    for kc in range(nkv):
                        i = kc - ndg * qt
                        c0 = max(0, i) * 128  # valid cols [c0:QT)
                        sps = p_S.tile([128, QT], F32, tag="S")
                        nc.tensor.matmul(
                            sps[:, c0:QT], kh[:, kc * 128:(kc + 1) * 128],
                            qh[:, qt * QT + c0:(qt + 1) * QT],
                            start=True, stop=True,
                        )
                        es = p_es.tile([128, QT], MM_DT, tag="es")
                        nc.scalar.activation(
                            es[:, c0:QT], sps[:, c0:QT],
                            mybir.ActivationFunctionType.Exp, scale=scale,
                        )
                        if i >= 0:  # triangular self-block
                            nc.vector.tensor_mul(
                                es[:, c0:c0 + 128], es[:, c0:c0 + 128],
                                tri_sb[:],
                            )
                        if kc == 0:
                            nc.vector.tensor_copy(esA[:], es[:])
                        elif kc % 2 == 0:
                            nc.vector.tensor_add(
                                esA[:, c0:QT], esA[:, c0:QT], es[:, c0:QT])
                        elif esB_c0 is None:
                            esB_c0 = c0
                            nc.gpsimd.tensor_copy(esB[:, c0:QT], es[:, c0:QT])
                        else:
                            nc.gpsimd.tensor_add(
                                esB[:, c0:QT], esB[:, c0:QT], es[:, c0:QT])
                        es_list.append((es, c0))
                        drain(rate)
                    if state["pend"] is not None:
                        flush_pend()
                    yps = p_yps.tile([128, QT], F32, tag="y")
                    for kc, (es, c0) in enumerate(es_list):
                        nc.tensor.matmul(
                            yps[:, c0:QT],
                            v_sb[:, kc, h * 128:(h + 1) * 128],
                            es[:, c0:QT],
                            start=(kc == 0), stop=(kc == nkv - 1),
                        )
                        drain(rate)
                    state["pend"] = (h, qsl, yps, esA, esB, esB_c0 or 0)

            # ---------- phase C emission, as a drainable stream ----------
            def phc_stream(p_ost):
                gidx = 0
                for qtp in range(nqt // 2):
                    for oc in range(ncc):
                        ops = [p_ps.tile([128, QT], F32, tag="mm",
                                         name=f"opc{gidx}_{j}")
                               for j in range(2)]

                        def grp(ops=ops, oc=oc, qtp=qtp, gidx=gidx):
                            for hh in range(Hc):
                                for j in range(2):
                                    qt = qtp * 2 + j
                                    nc.tensor.matmul(
                                        ops[j][:], wp_sb[:, hh, oc, :],
                                        y_sb[:, hh, qt * QT:(qt + 1) * QT],
                                        start=(hh == 0), stop=(hh == Hc - 1),
                                    )
                            for j in range(2):
                                qt = qtp * 2 + j
                                ost = p_ost.tile(
                                    [128, QT], BF16 if po_bf16 else F32,
                                    tag="ost")
                                if gidx % 2 == 0:
                                    nc.vector.tensor_copy(ost[:], ops[j][:])
                                else:
                                    nc.scalar.copy(ost[:], ops[j][:])
                                seng = (nc.sync if (gidx + j) % 2 == 0
                                        else nc.gpsimd)
                                seng.dma_start(
                                    po[oc * 128:(oc + 1) * 128,
                                       qt * QT:(qt + 1) * QT], ost[:])
                        yield grp
                        gidx += 1

            w_tiles = {}
            qk_tiles = {}

            def fetch_w(hh):
                if hh >= Hc:
                    return
                wqh = p_w.tile([128, ncc, 128], MM_DT, tag="wqh")
                nc.sync.dma_start(wqh[:], Wq[:, hh])
                wkh = p_w.tile([128, ncc, 128], MM_DT, tag="wkh")
                nc.sync.dma_start(wkh[:], Wk[:, hh])
                w_tiles[hh] = (wqh, wkh)

            with (
                tc.tile_pool(name="xw", bufs=1) as p_xw,
                tc.tile_pool(name="rope", bufs=2) as p_rope,
            ):
                # ---- DMAs: x token-chunked so v-proj starts right away ----
                wv_sb = p_xw.tile([128, ncc, Hc * 128], MM_DT, tag="wv")
                qv = ncc // 4
                for i in range(4):
                    nc.scalar.dma_start(wv_sb[:, i * qv:(i + 1) * qv, :],
                                        Wv[:, i * qv:(i + 1) * qv, :])
                x_sb = p_xw.tile([128, ncc, T], MM_DT, tag="x")
                ntk = 8
                tkw = T // ntk
                for tk in range(ntk):
                    eng = nc.sync if tk % 2 == 0 else nc.scalar
                    eng.dma_start(
                        x_sb[:, :, tk * tkw:(tk + 1) * tkw],
                        xT[:, :, tk * tkw:(tk + 1) * tkw])
                nc.sync.dma_start(cc_sb[:], cc[:])
                nc.sync.dma_start(ss_sb[:], ss[:])
                nc.sync.dma_start(tri_sb[:], tri[:])
                nc.sync.dma_start(ones_sb[:], ones[:])

                # ---- v projection (tokens on partitions), block pairs ----
                for pr in range(nkc // 2):
                    vps = [p_ps.tile([128, Hc * 128], F32, tag="mm",
                                     name=f"vps{j}") for j in range(2)]
                    for c in range(ncc):
                        for j in range(2):
                            tb = pr * 2 + j
                            nc.tensor.matmul(
                                vps[j][:], x_sb[:, c, tb * 128:(tb + 1) * 128],
                                wv_sb[:, c, :],
                                start=(c == 0), stop=(c == ncc - 1),
                            )
                    for j in range(2):
                        nc.vector.tensor_copy(v_sb[:, pr * 2 + j, :], vps[j][:])

                def rope(dst_ap, src_ps, cc_t, ss_t):
                    """dst = src*cc + swap64(src)*ss; src stays in PSUM."""
                    sw = p_rope.tile([128, QT], BF16, tag="rp_sw")
                    nc.scalar.copy(sw[0:64, :], src_ps[64:128, :])
                    nc.scalar.copy(sw[64:128, :], src_ps[0:64, :])
                    nc.gpsimd.tensor_mul(sw[:], sw[:], ss_t)
                    nc.vector.tensor_mul(dst_ap, src_ps[:], cc_t)
                    nc.vector.tensor_add(dst_ap, dst_ap, sw[:])

                def proj_stream(hh):
                    """Yield emit-callbacks for head hh's k/q projection."""
                    if hh >= Hc:
                        return
                    qh = p_qk.tile([128, T], MM_DT, tag="qh", name=f"qh{hh}")
                    kh = p_qk.tile([128, T], MM_DT, tag="kh", name=f"kh{hh}")
                    qk_tiles[hh] = (qh, kh)
                    wqh, wkh = w_tiles.pop(hh)
                    for w_sb, dst in ((wkh, kh), (wqh, qh)):
                        for pr in range(nqt // 2):
                            ps = [p_ps.tile([128, QT], F32, tag="mm",
                                            name=f"ps{hh}_{j}")
                                  for j in range(2)]
                            for c in range(ncc):
                                for j in range(2):
                                    tt = pr * 2 + j

                                    def mm(w_sb=w_sb, ps_t=ps[j], c=c, tt=tt):
                                        nc.tensor.matmul(
                                            ps_t[:], w_sb[:, c, :],
                                            x_sb[:, c,
                                                 tt * QT:(tt + 1) * QT],
                                            start=(c == 0),
                                            stop=(c == ncc - 1),
                                        )
                                    yield mm
                            for j in range(2):
                                tt = pr * 2 + j
                                sl = slice(tt * QT, (tt + 1) * QT)

                                def rp(dst=dst, ps_t=ps[j], sl=sl):
                                    rope(dst[:, sl], ps_t,
                                         cc_sb[:, sl], ss_sb[:, sl])
                                yield rp

                fetch_w(0)
                fetch_w(1)
                for fn in proj_stream(0):
                    fn()

                for h in range(Hc - 1):
                    fetch_w(h + 2)
                    pstream = proj_stream(h + 1)
                    qh, kh = qk_tiles[h]
                    attn_head(h, qh, kh, pstream, rate)
                    for fn in pstream:  # leftover proj of h+1
                        fn()

            # x/wv/rope freed; last head's attention drains phase C qtp0
            with tc.tile_pool(name="ost", bufs=6) as p_ost:
                for chk in range(4):
                    nc.sync.dma_start(
                        wp_sb[:, :, chk * 4:(chk + 1) * 4, :],
                        Wp[:, :, chk * 4:(chk + 1) * 4, :])
                phcs = phc_stream(p_ost)
                h = Hc - 1
                qh, kh = qk_tiles[h]
                # Only the ncc qtp0 groups are dep-safe during the last
                # head's attention (they read y[*, qt0/qt1] only), and only
                # once the (h3, qt1) flush has been EMITTED — i.e. from
                # qt2's PV phase onward. Anything more would enqueue a PE
                # matmul that waits on a flush emitted later in the same
                # in-order queue (deadlock).
                credit = [0.0]
                emitted = [0]

                def fill(n, qt, pv):
                    if not (qt > 2 or (qt == 2 and pv)):
                        return
                    if emitted[0] >= ncc:
                        return
                    credit[0] += n
                    while credit[0] >= 1.0 and emitted[0] < ncc:
                        fn = next(phcs, None)
                        if fn is None:
                            credit[0] = 0.0
                            return
                        fn()
                        emitted[0] += 1
                        credit[0] -= 1.0

                # inline attention for the last head with phC filler
                for qt in range(nqt):
                    nkv = ndg * (qt + 1)
                    qsl = slice(qt * QT, (qt + 1) * QT)
                    esA = p_dn.tile([128, QT], F32, tag="esA")
                    esB = p_dn.tile([128, QT], MM_DT, tag="esB")
                    esB_c0 = None
                    es_list = []
                    for kc in range(nkv):
                        i = kc - ndg * qt
                        c0 = max(0, i) * 128
                        sps = p_S.tile([128, QT], F32, tag="S")
                        nc.tensor.matmul(
                            sps[:, c0:QT], kh[:, kc * 128:(kc + 1) * 128],
                            qh[:, qt * QT + c0:(qt + 1) * QT],
                            start=True, stop=True,
                        )
                        es = p_es.tile([128, QT], MM_DT, tag="es")
                        nc.scalar.activation(
                            es[:, c0:QT], sps[:, c0:QT],
                            mybir.ActivationFunctionType.Exp, scale=scale,
                        )
                        if i >= 0:
                            nc.vector.tensor_mul(
                                es[:, c0:c0 + 128], es[:, c0:c0 + 128],
                                tri_sb[:],
                            )
                        if kc == 0:
                            nc.vector.tensor_copy(esA[:], es[:])
                        elif kc % 2 == 0:
                            nc.vector.tensor_add(
                                esA[:, c0:QT], esA[:, c0:QT], es[:, c0:QT])
                        elif esB_c0 is None:
                            esB_c0 = c0
                            nc.gpsimd.tensor_copy(esB[:, c0:QT], es[:, c0:QT])
                        else:
                            nc.gpsimd.tensor_add(
                                esB[:, c0:QT], esB[:, c0:QT], es[:, c0:QT])
                        es_list.append((es, c0))
                        fill(0.5, qt, False)
                    if state["pend"] is not None:
                        flush_pend()
                    yps = p_yps.tile([128, QT], F32, tag="y")
                    for kc, (es, c0) in enumerate(es_list):
                        nc.tensor.matmul(
                            yps[:, c0:QT],
                            v_sb[:, kc, h * 128:(h + 1) * 128],
                            es[:, c0:QT],
                            start=(kc == 0), stop=(kc == nkv - 1),
                        )
                        fill(0.5, qt, True)
                    state["pend"] = (h, qsl, yps, esA, esB, esB_c0 or 0)
                flush_pend()
                state["pend"] = None

                # ---- rest of the partial output projection ----
                for fn in phcs:
                    fn()

    nc.compile()
    return nc


def _prep_inputs(x, W_attn, W_proj, rope_cos, rope_sin, B, T, C):
    import ml_dtypes
    mmnp = ml_dtypes.bfloat16
    D = 128
    H = C // D
    Hc = H * B // N_CORES
    ncc = C // 128

    perm = np.concatenate([np.arange(0, D, 2), np.arange(1, D, 2)])
    cosT = rope_cos.T.astype(np.float32)
    sinT = rope_sin.T.astype(np.float32)
    cc = np.concatenate([cosT, cosT], axis=0).astype(mmnp)
    ss = np.concatenate([-sinT, sinT], axis=0).astype(mmnp)

    tri = (np.arange(128)[:, None] <= np.arange(128)[None, :]).astype(mmnp)
    trib = np.where(np.arange(128)[:, None] <= np.arange(128)[None, :],
                    0.0, -1e30).astype(np.float32)
    ones = np.ones((128, 128), dtype=mmnp)

    xTs = [
        np.ascontiguousarray(
            x[b].T.reshape(ncc, 128, T).transpose(1, 0, 2)).astype(mmnp)
        for b in range(B)
    ]

    def stat_tiles(w):  # [C, Hc*128] -> [128, Hc, ncc, 128]
        return np.ascontiguousarray(
            w.reshape(ncc, 128, Hc, 128).transpose(1, 2, 0, 3)).astype(mmnp)

    groups = []
    for g in range(N_CORES // B):
        hsl = np.arange(g * Hc * D, (g + 1) * Hc * D)
        cperm = np.concatenate([g * Hc * D + h * D + perm for h in range(Hc)])
        Wq_t = stat_tiles(W_attn[:, 0:C][:, cperm])
        Wk_t = stat_tiles(W_attn[:, C:2 * C][:, cperm])
        Wv_t = np.ascontiguousarray(
            W_attn[:, 2 * C:3 * C][:, hsl]
            .reshape(ncc, 128, Hc * 128).transpose(1, 0, 2)).astype(mmnp)
        Wp_t = np.ascontiguousarray(
            W_proj[hsl, :].reshape(Hc, 128, ncc, 128)
            .transpose(1, 0, 2, 3)).astype(mmnp)
        groups.append((Wq_t, Wk_t, Wv_t, Wp_t))

    in_maps = []
    for m in range(N_CORES):
        b = m // (N_CORES // B)
        Wq_t, Wk_t, Wv_t, Wp_t = groups[m % (N_CORES // B)]
        in_maps.append({
            "xT": xTs[b], "Wq": Wq_t, "Wk": Wk_t, "Wv": Wv_t, "Wp": Wp_t,
            "cc": cc, "ss": ss, "tri": tri, "trib": trib, "ones": ones,
        })
    return in_maps


_NC_CACHE = {}


def run(x, W_attn, W_proj, rope_cos, rope_sin, attention_mask=None, trace=False):
    B, T, C = x.shape
    key = (B, T, C)
    if key not in _NC_CACHE:
        _NC_CACHE[key] = build_nc(B, T, C)
    nc = _NC_CACHE[key]
    in_maps = _prep_inputs(
        np.asarray(x, dtype=np.float32),
        np.asarray(W_attn, dtype=np.float32),
        np.asarray(W_proj, dtype=np.float32),
        np.asarray(rope_cos, dtype=np.float32),
        np.asarray(rope_sin, dtype=np.float32),
        B, T, C,
    )
    res = run_bass_kernel_spmd(nc, in_maps, list(range(N_CORES)), trace=trace)
    gpb = N_CORES // B
    out = np.empty((B, T, C), dtype=np.float32)
    for b in range(B):
        acc = res.results[b * gpb]["po"].astype(np.float64)
        for j in range(1, gpb):
            acc += res.results[b * gpb + j]["po"]
        out[b] = acc.T
    return out, res


def kernel(x, W_attn, W_proj, rope_cos, rope_sin, attention_mask):
    out, _ = run(x, W_attn, W_proj, rope_cos, rope_sin)
    return out
